# revision 1
# baseline (speedup 1.0000x reference)
"""Trainium2 Bass kernel for BinaryTimedPSP (causal boxcar window sum + clip).

psp[t] = clip(sum_{k=max(0,t-D+1)}^{t} x[k], 0, 1) along time axis of a
[T=2048, B=16, N=2048] f32 spike tensor, D = duration (100).

Strategy: pure data-parallel over the 8 NeuronCores — the flattened B*N axis
(32768 columns) is split into 8 slabs of 4096 columns. Each core processes a
[T, 4096] slab:
  - time is tiled into 16 chunks of 128 rows, loaded as [128 part, 4096 free]
  - the window sum of chunk i is a block-banded matmul:
      out_i = A_0 @ x_i + A_1 @ x_{i-1} (+ ... for D > 128)
    where A_m[r, c] = 1 iff 0 <= (r + 128*m) - c < D. Exact for 0/1 spikes.
  - matmuls run as float32r (1 cycle/row at N=512) accumulating in f32 PSUM
  - clip to [0,1] == min(., 1.0) since the sum is >= 0, fused into the
    PSUM->SBUF copy on the vector engine
No cross-core communication; the gather is a host-side concatenate.
"""

import numpy as np

T_FULL, B_FULL, N_FULL = 2048, 16, 2048
NCORES = 8
P = 128
COLS = B_FULL * N_FULL          # 32768
FREE = COLS // NCORES           # 4096 columns per core
NCHUNK = T_FULL // P            # 16 time chunks
FTILE = 512                     # one PSUM bank of f32
NFT = FREE // FTILE             # 8

_CACHE: dict = {}


def _n_mats(d: int) -> int:
    # number of 128x128 band blocks: block m covers lags [128m-127, 128m+127]
    n = (d + P - 2) // P + 1 if d > 1 else 1
    n = max(1, min(n, NCHUNK))
    # exact condition: include m while 128m - 127 <= d - 1
    n = 1
    while P * n - (P - 1) <= d - 1 and n < NCHUNK:
        n += 1
    return n


def _weights(d: int, n_mats: int) -> np.ndarray:
    # W[m*128 + c, r] = A_m[r, c] = 1 iff 0 <= (r + 128m) - c < d
    # (lhsT layout: partition dim = contraction c, free dim = output row r)
    r = np.arange(P)[None, :]
    c = np.arange(P)[:, None]
    mats = []
    for m in range(n_mats):
        diff = r + P * m - c
        mats.append(((diff >= 0) & (diff < d)).astype(np.float32))
    return np.concatenate(mats, axis=0)


def _build(d: int):
    import concourse.bacc as bacc
    import concourse.mybir as mybir
    from concourse.tile import TileContext

    n_mats = _n_mats(d)
    f32 = mybir.dt.float32
    f32r = mybir.dt.float32r
    f8 = mybir.dt.float8e4

    nc = bacc.Bacc(None)
    x = nc.dram_tensor("x", [T_FULL, FREE], f32r, kind="ExternalInput")
    w = nc.dram_tensor("w", [n_mats * P, P], f32r, kind="ExternalInput")
    # outputs are exactly {0.0, 1.0} (integer boxcar sum clipped), so fp8e4
    # is a bit-exact representation — quarters the store traffic; the host
    # gather restores float32 losslessly
    y = nc.dram_tensor("y", [T_FULL, FREE], f8, kind="ExternalOutput")
    xr = x.rearrange("(n p) f -> n p f", p=P)
    yr = y.rearrange("(n p) f -> n p f", p=P)
    wr = w.rearrange("(m p) q -> m p q", p=P)

    with nc.allow_low_precision("output values are exactly 0/1; fp8e4 is lossless"), TileContext(nc) as tc:
        with (
            tc.tile_pool(name="wpool", bufs=1) as wpool,
            tc.tile_pool(name="xpool", bufs=4) as xpool,
            tc.tile_pool(name="opool", bufs=3) as opool,
            tc.tile_pool(name="ppool", bufs=8, space="PSUM") as ppool,
        ):
            # first x chunk goes out before the (tiny) weight DMAs so the
            # SDMA engines ramp on real work immediately
            xs = []
            x0 = xpool.tile([P, FREE], f32r, tag="x")
            nc.sync.dma_start(out=x0, in_=xr[0])
            xs.append(x0)
            wts = []
            for m in range(n_mats):
                wt = wpool.tile([P, P], f32r, tag=f"w{m}")
                nc.sync.dma_start(out=wt, in_=wr[m])
                wts.append(wt)
            for i in range(NCHUNK):
                if i > 0:
                    xt = xpool.tile([P, FREE], f32r, tag="x")
                    nc.sync.dma_start(out=xt, in_=xr[i])
                    xs.append(xt)
                ot = opool.tile([P, FREE], f8)
                terms = [m for m in range(n_mats) if i - m >= 0]
                for f in range(NFT):
                    ps = ppool.tile([P, FTILE], f32)
                    fs = slice(f * FTILE, (f + 1) * FTILE)
                    for j, m in enumerate(terms):
                        nc.tensor.matmul(
                            ps,
                            wts[m],
                            xs[i - m][:, fs],
                            start=(j == 0),
                            stop=(j == len(terms) - 1),
                        )
                    nc.vector.tensor_scalar_min(out=ot[:, fs], in0=ps, scalar1=1.0)
                # separate HWDGE ring (scalar) for stores so loads (sync ring)
                # and stores interleave at the SDMA packet level; the last
                # chunk's store goes out in halves so the tail drains while
                # the final DVE tiles finish
                if i == NCHUNK - 1:
                    half = FREE // 2
                    nc.scalar.dma_start(out=yr[i][:, :half], in_=ot[:, :half])
                    nc.scalar.dma_start(out=yr[i][:, half:], in_=ot[:, half:])
                else:
                    nc.scalar.dma_start(out=yr[i], in_=ot)
    nc.finalize()
    return nc, n_mats


def _get_built(d: int):
    if d not in _CACHE:
        _CACHE[d] = _build(d)
    return _CACHE[d]


def kernel(input_spikes, duration, _trace=False):
    from concourse.bass_utils import run_bass_kernel_spmd

    x = np.ascontiguousarray(np.asarray(input_spikes, dtype=np.float32))
    d = int(duration)
    assert x.shape == (T_FULL, B_FULL, N_FULL), x.shape

    nc, n_mats = _get_built(d)
    W = _weights(d, n_mats)

    xf = x.reshape(T_FULL, COLS)
    in_maps = [
        {"x": np.ascontiguousarray(xf[:, c * FREE : (c + 1) * FREE]), "w": W}
        for c in range(NCORES)
    ]
    res = run_bass_kernel_spmd(
        nc, in_maps, core_ids=list(range(NCORES)), trace=_trace
    )
    out = np.concatenate([r["y"] for r in res.results], axis=1)
    out = out.astype(np.float32).reshape(T_FULL, B_FULL, N_FULL)
    if _trace:
        return out, res
    return out



# revision 3
# speedup vs baseline: 1.6296x; 1.6296x over previous
"""Trainium2 Bass kernel for BinaryTimedPSP (causal boxcar window sum + clip).

psp[t] = clip(sum_{k=max(0,t-D+1)}^{t} x[k], 0, 1) along time axis of a
[T=2048, B=16, N=2048] f32 spike tensor, D = duration (100).

Strategy (v2): pure data-parallel over 8 NeuronCores; each core owns a
[T, 4096] slab of the flattened B*N axis.
  - input is cast to fp8e4 on the host (0/1 values are exact) -> 4x less
    HBM read traffic than f32
  - the whole slab lives in SBUF as one [128, 16, 4096] tile; time chunk i
    is written by its own DMA, and the window sum of chunk i is ONE
    DoubleRow fp8 matmul with K=256: ktile0 = chunk i-1 (band block A1),
    ktile1 = chunk i (band block A0). Chunk 0 uses a weight tile whose
    second k-tile is zero. 0.5 cycles/row = 4x the f32r matmul rate.
  - PSUM eviction (the clip) is split across two engines: DVE does
    tensor_scalar_min(.,1) on the low half, Act does activation Sign on
    the high half (sums are >= 0 so sign(s) == min(s,1) exactly).
  - outputs are exactly {0,1} so fp8e4 stores are bit-exact; the host
    gather restores f32 losslessly.
No cross-core communication; the gather is a host-side concatenate.
"""

import numpy as np

T_FULL, B_FULL, N_FULL = 2048, 16, 2048
NCORES = 8
P = 128
COLS = B_FULL * N_FULL          # 32768
FREE = COLS // NCORES           # 4096 columns per core
NCHUNK = T_FULL // P            # 16 time chunks
EV = 2048                       # eviction tile: 4 PSUM banks of f32
FTILE = 512                     # one PSUM bank of f32 (matmul out width)

_CACHE: dict = {}


def _band_weights(d: int) -> np.ndarray:
    """[2, 128, 2, 128] fp8 lhsT weights: [which, c(part), ktile, r(free)].

    which=0 (main, chunks i>=1): ktile0 = A1^T (applies to chunk i-1),
                                 ktile1 = A0^T (chunk i)
    which=1 (first, chunk 0):    ktile0 = A0^T (chunk 0), ktile1 = 0
    A0[r,c] = 1 iff 0 <= r-c < d ;  A1[r,c] = 1 iff 0 <= r+128-c < d
    """
    import ml_dtypes

    r = np.arange(P)[None, :]
    c = np.arange(P)[:, None]
    a0t = ((r - c >= 0) & (r - c < d)).astype(np.float32)        # [c, r]
    a1t = ((r + P - c >= 0) & (r + P - c < d)).astype(np.float32)
    w = np.zeros((2, P, 2, P), np.float32)
    w[0, :, 0, :] = a1t
    w[0, :, 1, :] = a0t
    w[1, :, 0, :] = a0t
    return w.astype(ml_dtypes.float8_e4m3)


def _build(d: int):
    import concourse.bacc as bacc
    import concourse.mybir as mybir
    from concourse.tile import TileContext

    f32 = mybir.dt.float32
    f8 = mybir.dt.float8e4
    DR = mybir.MatmulPerfMode.DoubleRow
    Sign = mybir.ActivationFunctionType.Sign

    nc = bacc.Bacc(None)
    x = nc.dram_tensor("x", [T_FULL, FREE], f8, kind="ExternalInput")
    w = nc.dram_tensor("w", [2 * P, 2 * P], f8, kind="ExternalInput")
    y = nc.dram_tensor("y", [T_FULL, FREE], f8, kind="ExternalOutput")
    xr = x.rearrange("(n p) f -> n p f", p=P)
    yr = y.rearrange("(n p) f -> n p f", p=P)
    wr = w.rearrange("(m p) (k r) -> m p k r", p=P, k=2)

    with nc.allow_low_precision("values are exactly 0/1; fp8e4 is lossless"), TileContext(nc) as tc:
        with (
            tc.tile_pool(name="wpool", bufs=1) as wpool,
            tc.tile_pool(name="xpool", bufs=1) as xpool,
            tc.tile_pool(name="opool", bufs=3) as opool,
            tc.tile_pool(name="ppool", bufs=2, space="PSUM") as ppool,
        ):
            # weights first (tiny), then every chunk load up-front on the
            # sync ring; the slab is persistent so loads have no hazards
            wm = wpool.tile([P, 2, P], f8, tag="wm")
            wf = wpool.tile([P, 2, P], f8, tag="wf")
            nc.sync.dma_start(out=wm, in_=wr[0])
            nc.sync.dma_start(out=wf, in_=wr[1])
            slab = xpool.tile([P, NCHUNK, FREE], f8, tag="slab")
            for i in range(NCHUNK):
                nc.sync.dma_start(out=slab[:, i, :], in_=xr[i])

            for i in range(NCHUNK):
                # rhs k-tile pair: (chunk i-1, chunk i); chunk 0 pairs with
                # chunk 1 but its weight k-tile1 is zero so the value is
                # ignored (only adds a dep on load 1, which is early anyway)
                lo = i - 1 if i > 0 else 0
                wt = wm if i > 0 else wf
                ot = opool.tile([P, FREE], f8, tag="o")
                for h in range(2):
                    ps = ppool.tile([P, EV], f32, tag="ps")
                    for f in range(EV // FTILE):
                        cs = h * EV + f * FTILE
                        nc.tensor.matmul(
                            ps[:, f * FTILE : (f + 1) * FTILE],
                            wt,
                            slab[:, lo : lo + 2, cs : cs + FTILE],
                            start=True,
                            stop=True,
                            perf_mode=DR,
                        )
                    if h == 0:
                        # DVE evicts the low half with the clip fused
                        nc.vector.tensor_scalar_min(
                            out=ot[:, 0:EV], in0=ps, scalar1=1.0
                        )
                        nc.sync.dma_start(out=yr[i][:, 0:EV], in_=ot[:, 0:EV])
                    else:
                        # Act evicts the high half: sums are >= 0 integers so
                        # sign(s) == min(s, 1) exactly
                        nc.scalar.activation(out=ot[:, EV : 2 * EV], in_=ps, func=Sign)
                        nc.scalar.dma_start(
                            out=yr[i][:, EV : 2 * EV], in_=ot[:, EV : 2 * EV]
                        )
    nc.finalize()
    return nc


def _get_built(d: int):
    if d not in _CACHE:
        _CACHE[d] = _build(d)
    return _CACHE[d]


def kernel(input_spikes, duration, _trace=False):
    import ml_dtypes
    from concourse.bass_utils import run_bass_kernel_spmd

    d = int(duration)
    # the fused DoubleRow band matmul covers windows up to 129 rows back
    assert 1 <= d <= P + 1, d
    x = np.asarray(input_spikes)
    assert x.shape == (T_FULL, B_FULL, N_FULL), x.shape

    nc = _get_built(d)
    W = _band_weights(d).reshape(2 * P, 2 * P)

    # exact host-side cast: spikes are {0.0, 1.0}; 1.0 in fp8e4m3 is 0x38
    f8 = ml_dtypes.float8_e4m3
    xb = (np.asarray(x, dtype=np.float32).reshape(T_FULL, COLS) != 0).astype(
        np.uint8
    ) * np.uint8(0x38)
    in_maps = [
        {
            "x": np.ascontiguousarray(xb[:, c * FREE : (c + 1) * FREE]).view(f8),
            "w": W,
        }
        for c in range(NCORES)
    ]
    res = run_bass_kernel_spmd(
        nc, in_maps, core_ids=list(range(NCORES)), trace=_trace
    )
    out = np.concatenate([r["y"] for r in res.results], axis=1)
    out = out.astype(np.float32).reshape(T_FULL, B_FULL, N_FULL)
    if _trace:
        return out, res
    return out


# revision 4
# speedup vs baseline: 1.9078x; 1.1707x over previous
"""Trainium2 Bass kernel for BinaryTimedPSP (causal boxcar window sum + clip).

psp[t] = clip(sum_{k=max(0,t-D+1)}^{t} x[k], 0, 1) along time axis of a
[T=2048, B=16, N=2048] f32 spike tensor, D = duration (100).

Strategy (v2): pure data-parallel over 8 NeuronCores; each core owns a
[T, 4096] slab of the flattened B*N axis.
  - input is cast to fp8e4 on the host (0/1 values are exact) -> 4x less
    HBM read traffic than f32
  - the whole slab lives in SBUF as one [128, 16, 4096] tile; time chunk i
    is written by its own DMA, and the window sum of chunk i is ONE
    DoubleRow fp8 matmul with K=256: ktile0 = chunk i-1 (band block A1),
    ktile1 = chunk i (band block A0). Chunk 0 uses a weight tile whose
    second k-tile is zero. 0.5 cycles/row = 4x the f32r matmul rate.
  - PSUM eviction (the clip) is split across two engines: DVE does
    tensor_scalar_min(.,1) on the low half, Act does activation Sign on
    the high half (sums are >= 0 so sign(s) == min(s,1) exactly).
  - outputs are exactly {0,1} so fp8e4 stores are bit-exact; the host
    gather restores f32 losslessly.
No cross-core communication; the gather is a host-side concatenate.
"""

import numpy as np

T_FULL, B_FULL, N_FULL = 2048, 16, 2048
NCORES = 8
P = 128
COLS = B_FULL * N_FULL          # 32768
FREE = COLS // NCORES           # 4096 columns per core
NCHUNK = T_FULL // P            # 16 time chunks
EV = 2048                       # eviction tile: 4 PSUM banks of f32
FTILE = 512                     # one PSUM bank of f32 (matmul out width)

_CACHE: dict = {}


def _band_weights(d: int) -> np.ndarray:
    """[2, 128, 2, 128] fp8 lhsT weights: [which, c(part), ktile, r(free)].

    which=0 (main, chunks i>=1): ktile0 = A1^T (applies to chunk i-1),
                                 ktile1 = A0^T (chunk i)
    which=1 (first, chunk 0):    ktile0 = A0^T (chunk 0), ktile1 = 0
    A0[r,c] = 1 iff 0 <= r-c < d ;  A1[r,c] = 1 iff 0 <= r+128-c < d
    """
    import ml_dtypes

    r = np.arange(P)[None, :]
    c = np.arange(P)[:, None]
    a0t = ((r - c >= 0) & (r - c < d)).astype(np.float32)        # [c, r]
    a1t = ((r + P - c >= 0) & (r + P - c < d)).astype(np.float32)
    w = np.zeros((2, P, 2, P), np.float32)
    w[0, :, 0, :] = a1t
    w[0, :, 1, :] = a0t
    w[1, :, 0, :] = a0t
    return w.astype(ml_dtypes.float8_e4m3)


def _build(d: int):
    import concourse.bacc as bacc
    import concourse.mybir as mybir
    from concourse.tile import TileContext

    f32 = mybir.dt.float32
    f8 = mybir.dt.float8e4
    DR = mybir.MatmulPerfMode.DoubleRow
    Sign = mybir.ActivationFunctionType.Sign

    nc = bacc.Bacc(None)
    x = nc.dram_tensor("x", [T_FULL, FREE], f8, kind="ExternalInput")
    w = nc.dram_tensor("w", [2 * P, 2 * P], f8, kind="ExternalInput")
    y = nc.dram_tensor("y", [T_FULL, FREE], f8, kind="ExternalOutput")
    xr = x.rearrange("(n p) f -> n p f", p=P)
    yr = y.rearrange("(n p) f -> n p f", p=P)
    wr = w.rearrange("(m p) (k r) -> m p k r", p=P, k=2)

    QV = 1024                   # psum tile: 2 banks of f32
    NQ = FREE // QV             # 4 psum tiles per chunk
    # chunks where DVE evicts only q0 and Act takes q1-q3, so the two
    # engines' total eviction time comes out balanced (DVE is slower/elem)
    ACT_HEAVY = (5, 10)

    with nc.allow_low_precision("values are exactly 0/1; fp8e4 is lossless"), TileContext(nc) as tc:
        with (
            tc.tile_pool(name="wpool", bufs=1) as wpool,
            tc.tile_pool(name="xpool", bufs=1) as xpool,
            tc.tile_pool(name="opool", bufs=3) as opool,
            tc.tile_pool(name="ppool", bufs=4, space="PSUM") as ppool,
        ):
            # weights first (tiny); chunks 0/1 loaded in interleaved column
            # strips so the first matmuls unblock early; remaining chunks
            # loaded whole. The slab is persistent so loads have no hazards.
            wm = wpool.tile([P, 2, P], f8, tag="wm")
            wf = wpool.tile([P, 2, P], f8, tag="wf")
            nc.sync.dma_start(out=wm, in_=wr[0])
            nc.sync.dma_start(out=wf, in_=wr[1])
            slab = xpool.tile([P, NCHUNK, FREE], f8, tag="slab")
            for s in range(4):
                for i in range(2):
                    nc.sync.dma_start(
                        out=slab[:, i, s * QV : (s + 1) * QV],
                        in_=xr[i][:, s * QV : (s + 1) * QV],
                    )
            for i in range(2, NCHUNK):
                nc.sync.dma_start(out=slab[:, i, :], in_=xr[i])

            for i in range(NCHUNK):
                # rhs k-tile pair: (chunk i-1, chunk i); chunk 0 pairs with
                # chunk 1 but its weight k-tile1 is zero so the value is
                # ignored (only adds a dep on load 1, which is early anyway)
                lo = i - 1 if i > 0 else 0
                wt = wm if i > 0 else wf
                last = i == NCHUNK - 1
                ot = opool.tile([P, FREE], f8, tag="o")
                for q in range(NQ):
                    ps = ppool.tile([P, QV], f32, tag="ps")
                    for f in range(QV // FTILE):
                        cs = q * QV + f * FTILE
                        nc.tensor.matmul(
                            ps[:, f * FTILE : (f + 1) * FTILE],
                            wt,
                            slab[:, lo : lo + 2, cs : cs + FTILE],
                            start=True,
                            stop=True,
                            perf_mode=DR,
                        )
                    cs = q * QV
                    on_dve = q < (1 if i in ACT_HEAVY else 2)
                    if last:
                        on_dve = q % 2 == 0
                    if on_dve:
                        # DVE evicts with the clip fused into the copy
                        nc.vector.tensor_scalar_min(
                            out=ot[:, cs : cs + QV], in0=ps, scalar1=1.0
                        )
                    else:
                        # Act evicts via Sign: sums are >= 0 integers so
                        # sign(s) == min(s, 1) exactly
                        nc.scalar.activation(
                            out=ot[:, cs : cs + QV], in_=ps, func=Sign
                        )
                    if last:
                        # drain the tail in quarters as each evict lands
                        nc.sync.dma_start(
                            out=yr[i][:, cs : cs + QV], in_=ot[:, cs : cs + QV]
                        )
                if not last:
                    nc.sync.dma_start(out=yr[i], in_=ot)
    nc.finalize()
    return nc


def _get_built(d: int):
    if d not in _CACHE:
        _CACHE[d] = _build(d)
    return _CACHE[d]


def kernel(input_spikes, duration, _trace=False):
    import ml_dtypes
    from concourse.bass_utils import run_bass_kernel_spmd

    d = int(duration)
    # the fused DoubleRow band matmul covers windows up to 129 rows back
    assert 1 <= d <= P + 1, d
    x = np.asarray(input_spikes)
    assert x.shape == (T_FULL, B_FULL, N_FULL), x.shape

    nc = _get_built(d)
    W = _band_weights(d).reshape(2 * P, 2 * P)

    # exact host-side cast: spikes are {0.0, 1.0}; 1.0 in fp8e4m3 is 0x38
    f8 = ml_dtypes.float8_e4m3
    xb = (np.asarray(x, dtype=np.float32).reshape(T_FULL, COLS) != 0).astype(
        np.uint8
    ) * np.uint8(0x38)
    in_maps = [
        {
            "x": np.ascontiguousarray(xb[:, c * FREE : (c + 1) * FREE]).view(f8),
            "w": W,
        }
        for c in range(NCORES)
    ]
    res = run_bass_kernel_spmd(
        nc, in_maps, core_ids=list(range(NCORES)), trace=_trace
    )
    out = np.concatenate([r["y"] for r in res.results], axis=1)
    out = out.astype(np.float32).reshape(T_FULL, B_FULL, N_FULL)
    if _trace:
        return out, res
    return out


# revision 5
# speedup vs baseline: 1.9807x; 1.0382x over previous
"""Trainium2 Bass kernel for BinaryTimedPSP (causal boxcar window sum + clip).

psp[t] = clip(sum_{k=max(0,t-D+1)}^{t} x[k], 0, 1) along time axis of a
[T=2048, B=16, N=2048] f32 spike tensor, D = duration (100).

Strategy (v2): pure data-parallel over 8 NeuronCores; each core owns a
[T, 4096] slab of the flattened B*N axis.
  - input is cast to fp8e4 on the host (0/1 values are exact) -> 4x less
    HBM read traffic than f32
  - the whole slab lives in SBUF as one [128, 16, 4096] tile; time chunk i
    is written by its own DMA, and the window sum of chunk i is ONE
    DoubleRow fp8 matmul with K=256: ktile0 = chunk i-1 (band block A1),
    ktile1 = chunk i (band block A0). Chunk 0 uses a weight tile whose
    second k-tile is zero. 0.5 cycles/row = 4x the f32r matmul rate.
  - PSUM eviction (the clip) is split across two engines: DVE does
    tensor_scalar_min(.,1) on the low half, Act does activation Sign on
    the high half (sums are >= 0 so sign(s) == min(s,1) exactly).
  - outputs are exactly {0,1} so fp8e4 stores are bit-exact; the host
    gather restores f32 losslessly.
No cross-core communication; the gather is a host-side concatenate.
"""

import numpy as np

T_FULL, B_FULL, N_FULL = 2048, 16, 2048
NCORES = 8
P = 128
COLS = B_FULL * N_FULL          # 32768
FREE = COLS // NCORES           # 4096 columns per core
NCHUNK = T_FULL // P            # 16 time chunks
EV = 2048                       # eviction tile: 4 PSUM banks of f32
FTILE = 512                     # one PSUM bank of f32 (matmul out width)

_CACHE: dict = {}


def _band_weights(d: int) -> np.ndarray:
    """[2, 128, 2, 128] fp8 lhsT weights: [which, c(part), ktile, r(free)].

    which=0 (main, chunks i>=1): ktile0 = A1^T (applies to chunk i-1),
                                 ktile1 = A0^T (chunk i)
    which=1 (first, chunk 0):    ktile0 = A0^T (chunk 0), ktile1 = 0
    A0[r,c] = 1 iff 0 <= r-c < d ;  A1[r,c] = 1 iff 0 <= r+128-c < d
    """
    import ml_dtypes

    r = np.arange(P)[None, :]
    c = np.arange(P)[:, None]
    a0t = ((r - c >= 0) & (r - c < d)).astype(np.float32)        # [c, r]
    a1t = ((r + P - c >= 0) & (r + P - c < d)).astype(np.float32)
    w = np.zeros((2, P, 2, P), np.float32)
    w[0, :, 0, :] = a1t
    w[0, :, 1, :] = a0t
    w[1, :, 0, :] = a0t
    return w.astype(ml_dtypes.float8_e4m3)


def _build(d: int):
    import concourse.bacc as bacc
    import concourse.mybir as mybir
    from concourse.tile import TileContext

    f32 = mybir.dt.float32
    f8 = mybir.dt.float8e4
    DR = mybir.MatmulPerfMode.DoubleRow
    Sign = mybir.ActivationFunctionType.Sign

    nc = bacc.Bacc(None)
    x = nc.dram_tensor("x", [T_FULL, FREE], f8, kind="ExternalInput")
    w = nc.dram_tensor("w", [2 * P, 2 * P], f8, kind="ExternalInput")
    y = nc.dram_tensor("y", [T_FULL, FREE], f8, kind="ExternalOutput")
    xr = x.rearrange("(n p) f -> n p f", p=P)
    yr = y.rearrange("(n p) f -> n p f", p=P)
    wr = w.rearrange("(m p) (k r) -> m p k r", p=P, k=2)

    QV = 1024                   # psum tile: 2 banks of f32
    NQ = FREE // QV             # 4 psum tiles per chunk
    # chunks where DVE evicts only q0 and Act takes q1-q3, so the two
    # engines' total eviction time comes out balanced (DVE is slower/elem)
    ACT_HEAVY = (5, 10)

    with nc.allow_low_precision("values are exactly 0/1; fp8e4 is lossless"), TileContext(nc) as tc:
        with (
            tc.tile_pool(name="wpool", bufs=1) as wpool,
            tc.tile_pool(name="xpool", bufs=1) as xpool,
            tc.tile_pool(name="opool", bufs=12) as opool,
            tc.tile_pool(name="ppool", bufs=4, space="PSUM") as ppool,
        ):
            # weights first (tiny); chunks 0/1 loaded in interleaved column
            # strips so the first matmuls unblock early; remaining chunks
            # loaded whole. The slab is persistent so loads have no hazards.
            wm = wpool.tile([P, 2, P], f8, tag="wm")
            wf = wpool.tile([P, 2, P], f8, tag="wf")
            nc.sync.dma_start(out=wm, in_=wr[0])
            nc.sync.dma_start(out=wf, in_=wr[1])
            slab = xpool.tile([P, NCHUNK, FREE], f8, tag="slab")
            for s in range(4):
                for i in range(2):
                    nc.sync.dma_start(
                        out=slab[:, i, s * QV : (s + 1) * QV],
                        in_=xr[i][:, s * QV : (s + 1) * QV],
                    )
            for i in range(2, NCHUNK):
                nc.sync.dma_start(out=slab[:, i, :], in_=xr[i])

            for i in range(NCHUNK):
                # rhs k-tile pair: (chunk i-1, chunk i); chunk 0 pairs with
                # chunk 1 but its weight k-tile1 is zero so the value is
                # ignored (only adds a dep on load 1, which is early anyway)
                lo = i - 1 if i > 0 else 0
                wt = wm if i > 0 else wf
                last = i == NCHUNK - 1
                ot = opool.tile([P, FREE], f8, tag="o")
                for q in range(NQ):
                    ps = ppool.tile([P, QV], f32, tag="ps")
                    for f in range(QV // FTILE):
                        cs = q * QV + f * FTILE
                        nc.tensor.matmul(
                            ps[:, f * FTILE : (f + 1) * FTILE],
                            wt,
                            slab[:, lo : lo + 2, cs : cs + FTILE],
                            start=True,
                            stop=True,
                            perf_mode=DR,
                        )
                    cs = q * QV
                    on_dve = q < (1 if i in ACT_HEAVY else 2)
                    if last:
                        on_dve = q % 2 == 0
                    if on_dve:
                        # DVE evicts with the clip fused into the copy
                        nc.vector.tensor_scalar_min(
                            out=ot[:, cs : cs + QV], in0=ps, scalar1=1.0
                        )
                    else:
                        # Act evicts via Sign: sums are >= 0 integers so
                        # sign(s) == min(s, 1) exactly
                        nc.scalar.activation(
                            out=ot[:, cs : cs + QV], in_=ps, func=Sign
                        )
                    if last:
                        # drain the tail in quarters as each evict lands
                        nc.sync.dma_start(
                            out=yr[i][:, cs : cs + QV], in_=ot[:, cs : cs + QV]
                        )
                if not last:
                    nc.sync.dma_start(out=yr[i], in_=ot)
    nc.finalize()
    return nc


def _get_built(d: int):
    if d not in _CACHE:
        _CACHE[d] = _build(d)
    return _CACHE[d]


def kernel(input_spikes, duration, _trace=False):
    import ml_dtypes
    from concourse.bass_utils import run_bass_kernel_spmd

    d = int(duration)
    # the fused DoubleRow band matmul covers windows up to 129 rows back
    assert 1 <= d <= P + 1, d
    x = np.asarray(input_spikes)
    assert x.shape == (T_FULL, B_FULL, N_FULL), x.shape

    nc = _get_built(d)
    W = _band_weights(d).reshape(2 * P, 2 * P)

    # exact host-side cast: spikes are {0.0, 1.0}; 1.0 in fp8e4m3 is 0x38
    f8 = ml_dtypes.float8_e4m3
    xb = (np.asarray(x, dtype=np.float32).reshape(T_FULL, COLS) != 0).astype(
        np.uint8
    ) * np.uint8(0x38)
    in_maps = [
        {
            "x": np.ascontiguousarray(xb[:, c * FREE : (c + 1) * FREE]).view(f8),
            "w": W,
        }
        for c in range(NCORES)
    ]
    res = run_bass_kernel_spmd(
        nc, in_maps, core_ids=list(range(NCORES)), trace=_trace
    )
    out = np.concatenate([r["y"] for r in res.results], axis=1)
    out = out.astype(np.float32).reshape(T_FULL, B_FULL, N_FULL)
    if _trace:
        return out, res
    return out


# revision 8
# speedup vs baseline: 2.3432x; 1.1830x over previous
"""Trainium2 Bass kernel for BinaryTimedPSP (causal boxcar window sum + clip).

psp[t] = clip(sum_{k=max(0,t-D+1)}^{t} x[k], 0, 1) along time axis of a
[T=2048, B=16, N=2048] f32 spike tensor, D = duration (100).

Strategy (v2): pure data-parallel over 8 NeuronCores; each core owns a
[T, 4096] slab of the flattened B*N axis.
  - input is cast to fp8e4 on the host (0/1 values are exact) -> 4x less
    HBM read traffic than f32
  - the whole slab lives in SBUF as one [128, 16, 4096] tile; time chunk i
    is written by its own DMA, and the window sum of chunk i is ONE
    DoubleRow fp8 matmul with K=256: ktile0 = chunk i-1 (band block A1),
    ktile1 = chunk i (band block A0). Chunk 0 uses a weight tile whose
    second k-tile is zero. 0.5 cycles/row = 4x the f32r matmul rate.
  - PSUM eviction (the clip) is split across two engines: DVE does
    tensor_scalar_min(.,1) on the low half, Act does activation Sign on
    the high half (sums are >= 0 so sign(s) == min(s,1) exactly).
  - outputs are exactly {0,1} so fp8e4 stores are bit-exact; the host
    gather restores f32 losslessly.
No cross-core communication; the gather is a host-side concatenate.
"""

import numpy as np

T_FULL, B_FULL, N_FULL = 2048, 16, 2048
NCORES = 8
P = 128
COLS = B_FULL * N_FULL          # 32768
FREE = COLS // NCORES           # 4096 columns per core
NCHUNK = T_FULL // P            # 16 time chunks
EV = 2048                       # eviction tile: 4 PSUM banks of f32
FTILE = 512                     # one PSUM bank of f32 (matmul out width)

_CACHE: dict = {}


def _band_weights(d: int) -> np.ndarray:
    """[2, 128, 2, 128] fp8 lhsT weights: [which, c(part), ktile, r(free)].

    which=0 (main, chunks i>=1): ktile0 = A1^T (applies to chunk i-1),
                                 ktile1 = A0^T (chunk i)
    which=1 (first, chunk 0):    ktile0 = A0^T (chunk 0), ktile1 = 0
    A0[r,c] = 1 iff 0 <= r-c < d ;  A1[r,c] = 1 iff 0 <= r+128-c < d
    """
    import ml_dtypes

    r = np.arange(P)[None, :]
    c = np.arange(P)[:, None]
    a0t = ((r - c >= 0) & (r - c < d)).astype(np.float32)        # [c, r]
    a1t = ((r + P - c >= 0) & (r + P - c < d)).astype(np.float32)
    w = np.zeros((2, P, 2, P), np.float32)
    w[0, :, 0, :] = a1t
    w[0, :, 1, :] = a0t
    w[1, :, 0, :] = a0t
    return w.astype(ml_dtypes.float8_e4m3)


def _build(d: int):
    import concourse.bacc as bacc
    import concourse.mybir as mybir
    from concourse.tile import TileContext

    f32 = mybir.dt.float32
    f8 = mybir.dt.float8e4
    DR = mybir.MatmulPerfMode.DoubleRow
    Sign = mybir.ActivationFunctionType.Sign

    nc = bacc.Bacc(None)
    x = nc.dram_tensor("x", [T_FULL, FREE], f8, kind="ExternalInput")
    w = nc.dram_tensor("w", [2 * P, 2 * P], f8, kind="ExternalInput")
    y = nc.dram_tensor("y", [T_FULL, FREE], f8, kind="ExternalOutput")
    xr = x.rearrange("(n p) f -> n p f", p=P)
    yr = y.rearrange("(n p) f -> n p f", p=P)
    wr = w.rearrange("(m p) (k r) -> m p k r", p=P, k=2)

    QV = 1024                   # psum tile: 2 banks of f32
    NQ = FREE // QV             # 4 psum tiles per chunk
    # chunks where Act evicts q0-q2 and DVE only q3, so the two engines'
    # total eviction time comes out balanced (DVE is slower per element)
    ACT_HEAVY = (5, 10)

    with nc.allow_low_precision("values are exactly 0/1; fp8e4 is lossless"), TileContext(nc) as tc:
        with (
            tc.tile_pool(name="wpool", bufs=1) as wpool,
            tc.tile_pool(name="xpool", bufs=1) as xpool,
            tc.tile_pool(name="opool", bufs=12) as opool,
            tc.tile_pool(name="ppool", bufs=4, space="PSUM") as ppool,
        ):
            # chunks 0/1 loaded in interleaved column strips so the first
            # matmuls unblock early; the first strip pair goes before the
            # (tiny) weight loads so the one-time DGE ring-init cost is paid
            # on data the pipeline needs first. Remaining chunks load whole.
            # The slab is persistent so loads have no hazards.
            wm = wpool.tile([P, 2, P], f8, tag="wm")
            wf = wpool.tile([P, 2, P], f8, tag="wf")
            slab = xpool.tile([P, NCHUNK, FREE], f8, tag="slab")
            for i in range(2):
                nc.sync.dma_start(out=slab[:, i, 0:QV], in_=xr[i][:, 0:QV])
            nc.sync.dma_start(out=wf, in_=wr[1])
            nc.sync.dma_start(out=wm, in_=wr[0])
            for s in range(1, 4):
                for i in range(2):
                    nc.sync.dma_start(
                        out=slab[:, i, s * QV : (s + 1) * QV],
                        in_=xr[i][:, s * QV : (s + 1) * QV],
                    )
            for i in range(2, NCHUNK):
                nc.sync.dma_start(out=slab[:, i, :], in_=xr[i])

            for i in range(NCHUNK):
                # rhs k-tile pair: (chunk i-1, chunk i); chunk 0 pairs with
                # chunk 1 but its weight k-tile1 is zero so the value is
                # ignored (only adds a dep on load 1, which is early anyway)
                lo = i - 1 if i > 0 else 0
                wt = wm if i > 0 else wf
                last = i == NCHUNK - 1
                ot = opool.tile([P, FREE], f8, tag="o")
                for q in range(NQ):
                    ps = ppool.tile([P, QV], f32, tag="ps")
                    for f in range(QV // FTILE):
                        cs = q * QV + f * FTILE
                        nc.tensor.matmul(
                            ps[:, f * FTILE : (f + 1) * FTILE],
                            wt,
                            slab[:, lo : lo + 2, cs : cs + FTILE],
                            start=True,
                            stop=True,
                            perf_mode=DR,
                        )
                    cs = q * QV
                    # Act owns the low columns (its act-table load finishes
                    # during DMA warmup, and the low strips arrive first)
                    on_dve = q >= (3 if i in ACT_HEAVY else 2)
                    if last:
                        on_dve = q % 2 == 1
                    if on_dve:
                        # DVE evicts with the clip fused into the copy
                        nc.vector.tensor_scalar_min(
                            out=ot[:, cs : cs + QV], in0=ps, scalar1=1.0
                        )
                    else:
                        # Act evicts via Sign: sums are >= 0 integers so
                        # sign(s) == min(s, 1) exactly
                        nc.scalar.activation(
                            out=ot[:, cs : cs + QV], in_=ps, func=Sign
                        )
                    if last:
                        # drain the tail in quarters as each evict lands
                        nc.sync.dma_start(
                            out=yr[i][:, cs : cs + QV], in_=ot[:, cs : cs + QV]
                        )
                if not last:
                    nc.sync.dma_start(out=yr[i], in_=ot)
    nc.finalize()
    return nc


def _get_built(d: int):
    if d not in _CACHE:
        _CACHE[d] = _build(d)
    return _CACHE[d]


def kernel(input_spikes, duration, _trace=False):
    import ml_dtypes
    from concourse.bass_utils import run_bass_kernel_spmd

    d = int(duration)
    # the fused DoubleRow band matmul covers windows up to 129 rows back
    assert 1 <= d <= P + 1, d
    x = np.asarray(input_spikes)
    assert x.shape == (T_FULL, B_FULL, N_FULL), x.shape

    nc = _get_built(d)
    W = _band_weights(d).reshape(2 * P, 2 * P)

    # exact host-side cast: spikes are {0.0, 1.0}; 1.0 in fp8e4m3 is 0x38
    f8 = ml_dtypes.float8_e4m3
    xb = (np.asarray(x, dtype=np.float32).reshape(T_FULL, COLS) != 0).astype(
        np.uint8
    ) * np.uint8(0x38)
    in_maps = [
        {
            "x": np.ascontiguousarray(xb[:, c * FREE : (c + 1) * FREE]).view(f8),
            "w": W,
        }
        for c in range(NCORES)
    ]
    res = run_bass_kernel_spmd(
        nc, in_maps, core_ids=list(range(NCORES)), trace=_trace
    )
    out = np.concatenate([r["y"] for r in res.results], axis=1)
    out = out.astype(np.float32).reshape(T_FULL, B_FULL, N_FULL)
    if _trace:
        return out, res
    return out


# revision 10
# speedup vs baseline: 2.3558x; 1.0054x over previous
"""Trainium2 Bass kernel for BinaryTimedPSP (causal boxcar window sum + clip).

psp[t] = clip(sum_{k=max(0,t-D+1)}^{t} x[k], 0, 1) along time axis of a
[T=2048, B=16, N=2048] f32 spike tensor, D = duration (100).

Strategy (v2): pure data-parallel over 8 NeuronCores; each core owns a
[T, 4096] slab of the flattened B*N axis.
  - input is cast to fp8e4 on the host (0/1 values are exact) -> 4x less
    HBM read traffic than f32
  - the whole slab lives in SBUF as one [128, 16, 4096] tile; time chunk i
    is written by its own DMA, and the window sum of chunk i is ONE
    DoubleRow fp8 matmul with K=256: ktile0 = chunk i-1 (band block A1),
    ktile1 = chunk i (band block A0). Chunk 0 uses a weight tile whose
    second k-tile is zero. 0.5 cycles/row = 4x the f32r matmul rate.
  - PSUM eviction (the clip) is split across two engines: DVE does
    tensor_scalar_min(.,1) on the low half, Act does activation Sign on
    the high half (sums are >= 0 so sign(s) == min(s,1) exactly).
  - outputs are exactly {0,1} so fp8e4 stores are bit-exact; the host
    gather restores f32 losslessly.
No cross-core communication; the gather is a host-side concatenate.
"""

import numpy as np

T_FULL, B_FULL, N_FULL = 2048, 16, 2048
NCORES = 8
P = 128
COLS = B_FULL * N_FULL          # 32768
FREE = COLS // NCORES           # 4096 columns per core
NCHUNK = T_FULL // P            # 16 time chunks
EV = 2048                       # eviction tile: 4 PSUM banks of f32
FTILE = 512                     # one PSUM bank of f32 (matmul out width)

_CACHE: dict = {}


def _band_weights(d: int) -> np.ndarray:
    """[2, 128, 2, 128] fp8 lhsT weights: [which, c(part), ktile, r(free)].

    which=0 (main, chunks i>=1): ktile0 = A1^T (applies to chunk i-1),
                                 ktile1 = A0^T (chunk i)
    which=1 (first, chunk 0):    ktile0 = A0^T (chunk 0), ktile1 = 0
    A0[r,c] = 1 iff 0 <= r-c < d ;  A1[r,c] = 1 iff 0 <= r+128-c < d
    """
    import ml_dtypes

    r = np.arange(P)[None, :]
    c = np.arange(P)[:, None]
    a0t = ((r - c >= 0) & (r - c < d)).astype(np.float32)        # [c, r]
    a1t = ((r + P - c >= 0) & (r + P - c < d)).astype(np.float32)
    w = np.zeros((2, P, 2, P), np.float32)
    w[0, :, 0, :] = a1t
    w[0, :, 1, :] = a0t
    w[1, :, 0, :] = a0t
    return w.astype(ml_dtypes.float8_e4m3)


def _build(d: int):
    import concourse.bacc as bacc
    import concourse.mybir as mybir
    from concourse.tile import TileContext

    f32 = mybir.dt.float32
    f8 = mybir.dt.float8e4
    DR = mybir.MatmulPerfMode.DoubleRow
    Sign = mybir.ActivationFunctionType.Sign

    nc = bacc.Bacc(None)
    x = nc.dram_tensor("x", [T_FULL, FREE], f8, kind="ExternalInput")
    w = nc.dram_tensor("w", [2 * P, 2 * P], f8, kind="ExternalInput")
    y = nc.dram_tensor("y", [T_FULL, FREE], f8, kind="ExternalOutput")
    xr = x.rearrange("(n p) f -> n p f", p=P)
    yr = y.rearrange("(n p) f -> n p f", p=P)
    wr = w.rearrange("(m p) (k r) -> m p k r", p=P, k=2)

    QV = 1024                   # psum tile: 2 banks of f32
    NQ = FREE // QV             # 4 psum tiles per chunk
    # chunks where Act evicts q0-q2 and DVE only q3, so the two engines'
    # total eviction time comes out balanced (DVE is slower per element)
    ACT_HEAVY = (5, 10)

    with nc.allow_low_precision("values are exactly 0/1; fp8e4 is lossless"), TileContext(nc) as tc:
        with (
            tc.tile_pool(name="wpool", bufs=1) as wpool,
            tc.tile_pool(name="xpool", bufs=1) as xpool,
            tc.tile_pool(name="opool", bufs=12) as opool,
            tc.tile_pool(name="ppool", bufs=4, space="PSUM") as ppool,
        ):
            # Per-DMA trigger cost (~0.6us on the sequencer) dominates small
            # transfers, so: chunks 0/1 as single whole loads (they gate the
            # first matmuls), then the rest as 1MB chunk-pair loads to halve
            # trigger serialization. Weights (tiny) go between. The slab is
            # persistent so loads have no hazards.
            wm = wpool.tile([P, 2, P], f8, tag="wm")
            wf = wpool.tile([P, 2, P], f8, tag="wf")
            slab = xpool.tile([P, NCHUNK, FREE], f8, tag="slab")
            nc.sync.dma_start(out=slab[:, 0, :], in_=xr[0])
            nc.sync.dma_start(out=slab[:, 1, :], in_=xr[1])
            nc.sync.dma_start(out=wf, in_=wr[1])
            nc.sync.dma_start(out=wm, in_=wr[0])
            xp = x.rearrange("(n p) f -> p n f", p=P)
            for i in range(2, NCHUNK, 2):
                nc.sync.dma_start(out=slab[:, i : i + 2, :], in_=xp[:, i : i + 2, :])

            for i in range(NCHUNK):
                # rhs k-tile pair: (chunk i-1, chunk i); chunk 0 pairs with
                # chunk 1 but its weight k-tile1 is zero so the value is
                # ignored (only adds a dep on load 1, which is early anyway)
                lo = i - 1 if i > 0 else 0
                wt = wm if i > 0 else wf
                last = i == NCHUNK - 1
                ot = opool.tile([P, FREE], f8, tag="o")
                for q in range(NQ):
                    ps = ppool.tile([P, QV], f32, tag="ps")
                    for f in range(QV // FTILE):
                        cs = q * QV + f * FTILE
                        nc.tensor.matmul(
                            ps[:, f * FTILE : (f + 1) * FTILE],
                            wt,
                            slab[:, lo : lo + 2, cs : cs + FTILE],
                            start=True,
                            stop=True,
                            perf_mode=DR,
                        )
                    cs = q * QV
                    # Act owns the low columns (its act-table load finishes
                    # during DMA warmup) -- except chunk 0, where DVE (no
                    # table load) takes the low half to start immediately
                    on_dve = q >= (3 if i in ACT_HEAVY else 2)
                    if i == 0:
                        on_dve = q < 2
                    if last:
                        on_dve = q % 2 == 1
                    if on_dve:
                        # DVE evicts with the clip fused into the copy
                        nc.vector.tensor_scalar_min(
                            out=ot[:, cs : cs + QV], in0=ps, scalar1=1.0
                        )
                    else:
                        # Act evicts via Sign: sums are >= 0 integers so
                        # sign(s) == min(s, 1) exactly
                        nc.scalar.activation(
                            out=ot[:, cs : cs + QV], in_=ps, func=Sign
                        )
                    if last:
                        # drain the tail in quarters as each evict lands
                        nc.sync.dma_start(
                            out=yr[i][:, cs : cs + QV], in_=ot[:, cs : cs + QV]
                        )
                if not last:
                    nc.sync.dma_start(out=yr[i], in_=ot)
    nc.finalize()
    return nc


def _get_built(d: int):
    if d not in _CACHE:
        _CACHE[d] = _build(d)
    return _CACHE[d]


def kernel(input_spikes, duration, _trace=False):
    import ml_dtypes
    from concourse.bass_utils import run_bass_kernel_spmd

    d = int(duration)
    # the fused DoubleRow band matmul covers windows up to 129 rows back
    assert 1 <= d <= P + 1, d
    x = np.asarray(input_spikes)
    assert x.shape == (T_FULL, B_FULL, N_FULL), x.shape

    nc = _get_built(d)
    W = _band_weights(d).reshape(2 * P, 2 * P)

    # exact host-side cast: spikes are {0.0, 1.0}; 1.0 in fp8e4m3 is 0x38
    f8 = ml_dtypes.float8_e4m3
    xb = (np.asarray(x, dtype=np.float32).reshape(T_FULL, COLS) != 0).astype(
        np.uint8
    ) * np.uint8(0x38)
    in_maps = [
        {
            "x": np.ascontiguousarray(xb[:, c * FREE : (c + 1) * FREE]).view(f8),
            "w": W,
        }
        for c in range(NCORES)
    ]
    res = run_bass_kernel_spmd(
        nc, in_maps, core_ids=list(range(NCORES)), trace=_trace
    )
    out = np.concatenate([r["y"] for r in res.results], axis=1)
    out = out.astype(np.float32).reshape(T_FULL, B_FULL, N_FULL)
    if _trace:
        return out, res
    return out
